# revision 57
# baseline (speedup 1.0000x reference)
"""Trainium2 Bass kernel for a sparse (sliding-window) attention layer.

Reference computation (B=2, S=2048, D=2048, H=16 heads, window=256, fp32):
    qp = q @ Wq + bq ; kp = k @ Wk + bk ; vp = v @ Wv + bv
    per-head scores with mask (0 <= q_idx - k_idx <= 256), softmax, ctx
    out = merge_heads(ctx) @ Wo + bo
    returns (out, kp, vp)

Sharding: 8 cores = 2 (batch) x 4 (head groups of 4 heads / 512 dims).
Each core computes its batch's projections for its 512 output dims
(transposed layout for q/k so attention feeds straight into the PE),
the windowed attention for its 4 heads, and a partial out-projection
(rows of Wo owned by its heads).  Host sums the 4 partial outputs per
batch (the "out_proj all-reduce") and concatenates kp/vp slices.

The kernel is a 4-round pipeline over 512-token seq chunks: each round
projects q/k/v for the chunk, runs the windowed attention for the
chunk's 4 query blocks on all 4 heads, and emits the chunk's partial
out-projection.  This keeps the PE array streaming continuously (no
HAM re-throttle) and spreads DVE/ACT/DMA work evenly.

Engine budget choices:
  - the sliding-window mask is added on the TENSOR engine (an extra
    PSUM-accumulating matmul with an identity stationary) instead of a
    DVE tensor_tensor, keeping the Vector engine off the critical path
  - the padded prob buffers (transposed probs for the ctx matmul) are
    zeroed ONCE: the pad blocks are never written by any round
  - kp/vp/pout DRAM outputs are bf16 (host upconverts); matmuls are
    bf16 with fp32 PSUM accumulation; softmax statistics stay fp32
"""

import os
import sys

import numpy as np

B = 2
S = 2048
D = 2048
GD = 512          # dims per core (4 heads x 128)
NH = 4            # heads per core
P = 128
WIN = 256         # sliding window
NDB = D // P      # 16 contraction blocks
SC = 512          # seq chunk (one pipeline round)
NSC = S // SC     # 4 rounds
NSB = S // P      # 16 seq blocks
SCALE = 1.0 / np.sqrt(P)

_CACHE = {}
LAST_RESULTS = None


def _mm_dtype_name():
    return os.environ.get("KERNEL_MM_DT", "bf16")


def _build_nc():
    sys.path.insert(0, "/opt/trn_rl_repo")
    import concourse.bass as bass  # noqa: F401
    import concourse.tile as tile
    from concourse import mybir, bacc
    from concourse.masks import make_identity
    from contextlib import ExitStack

    F32 = mybir.dt.float32
    CDT = mybir.dt.bfloat16 if _mm_dtype_name() == "bf16" else F32

    nc = bacc.Bacc("TRN2", target_bir_lowering=False, debug=False, num_devices=8)

    xq_T = nc.dram_tensor("xq_T", [D, S], CDT, kind="ExternalInput")
    xk_T = nc.dram_tensor("xk_T", [D, S], CDT, kind="ExternalInput")
    xv_T = nc.dram_tensor("xv_T", [D, S], CDT, kind="ExternalInput")
    wq = nc.dram_tensor("wq", [D, GD], CDT, kind="ExternalInput")
    wk = nc.dram_tensor("wk", [D, GD], CDT, kind="ExternalInput")
    wv = nc.dram_tensor("wv", [D, GD], CDT, kind="ExternalInput")
    wo = nc.dram_tensor("wo", [GD, D], CDT, kind="ExternalInput")
    bq2 = nc.dram_tensor("bq2", [GD], F32, kind="ExternalInput")
    bk2 = nc.dram_tensor("bk2", [GD], F32, kind="ExternalInput")
    bvb = nc.dram_tensor("bvb", [P, GD], F32, kind="ExternalInput")
    maskd = nc.dram_tensor("maskd", [P, 3 * P], CDT, kind="ExternalInput")

    kpT_o = nc.dram_tensor("kpT", [GD, S], CDT, kind="ExternalOutput")
    vp_o = nc.dram_tensor("vp", [S, GD], CDT, kind="ExternalOutput")
    pout_o = nc.dram_tensor("pout", [S, D], CDT, kind="ExternalOutput")

    # partition-major views of the DRAM operands
    xq_r = xq_T.ap().rearrange("(do p) s -> p do s", p=P)
    xk_r = xk_T.ap().rearrange("(do p) s -> p do s", p=P)
    xv_r = xv_T.ap().rearrange("(do p) s -> p do s", p=P)
    wq_r = wq.ap().rearrange("(do p) n -> p do n", p=P)
    wk_r = wk.ap().rearrange("(do p) n -> p do n", p=P)
    wv_r = wv.ap().rearrange("(do p) n -> p do n", p=P)
    wo_r = wo.ap().rearrange("(h p) n -> p h n", p=P)
    bq_r = bq2.ap().rearrange("(h p) -> p h", p=P)
    bk_r = bk2.ap().rearrange("(h p) -> p h", p=P)
    kpT_r = kpT_o.ap().rearrange("(h p) s -> h p s", p=P)

    AluOp = mybir.AluOpType
    ActFn = mybir.ActivationFunctionType

    NQ = 4            # weight/x sub-tiles (4 db blocks each)

    with tile.TileContext(nc) as tc, ExitStack() as top:
        const = top.enter_context(tc.tile_pool(name="const", bufs=1))
        junk = const.tile([P, SC], CDT, name="junk")
        nc.gpsimd.memset(junk[:], 0.125)
        ident = const.tile([P, P], CDT, name="ident")
        make_identity(nc, ident[:])
        mask_sb = const.tile([P, 3 * P], CDT, name="mask_sb")
        nc.sync.dma_start(mask_sb[:], maskd.ap())
        bq_sb = const.tile([P, NH], F32, name="bq_sb")
        nc.sync.dma_start(bq_sb[:], bq_r)
        bk_sb = const.tile([P, NH], F32, name="bk_sb")
        nc.sync.dma_start(bk_sb[:], bk_r)
        bvb_sb = const.tile([P, GD], F32, name="bvb_sb")
        nc.sync.dma_start(bvb_sb[:], bvb.ap())

        # weights: q/k split into 4 sub-tiles so the first matmuls can
        # start as soon as the first quarter + first x quarter land.
        # DMAs are emitted lazily (just before first use) so the DMA
        # queues service the startup-critical transfers first.
        wpool = top.enter_context(tc.tile_pool(name="wpool", bufs=1))
        wq_sb = [wpool.tile([P, NDB // NQ, GD], CDT, name=f"wq_sb{i}")
                 for i in range(NQ)]
        wk_sb = [wpool.tile([P, NDB // NQ, GD], CDT, name=f"wk_sb{i}")
                 for i in range(NQ)]
        wv_sb = wpool.tile([P, NDB, GD], CDT, name="wv_sb")
        wo_sb = wpool.tile([P, NH, D], CDT, name="wo_sb")

        # long-lived activations
        persist1 = top.enter_context(tc.tile_pool(name="persist1", bufs=1))
        qpT = [persist1.tile([P, S], CDT, name=f"qpT{h}") for h in range(NH)]
        kpT = [persist1.tile([P, S], CDT, name=f"kpT{h}") for h in range(NH)]
        persist2 = top.enter_context(tc.tile_pool(name="persist2", bufs=1))
        vpB = [persist2.tile([P, GD], CDT, name=f"vpB{sb}") for sb in range(NSB)]
        persist3 = top.enter_context(tc.tile_pool(name="persist3", bufs=1))
        ctxT = [persist3.tile([P, S], CDT, name=f"ctxT{h}") for h in range(NH)]

        # transposed-prob buffers: [key-block rel 0..5, query 0..511].
        # Pad blocks (rel<t or rel>t+2) are never written by any round,
        # so a single memset keeps them zero for the whole kernel.
        persist4 = top.enter_context(tc.tile_pool(name="persist4", bufs=1))
        pbufs = [persist4.tile([P, 6, SC], CDT, name=f"pbuf{i}") for i in range(4)]
        for pb in pbufs:
            nc.gpsimd.memset(pb[:], 0.0)

        # working pools
        xpool = top.enter_context(tc.tile_pool(name="xpool", bufs=2))
        wkp = top.enter_context(tc.tile_pool(name="wkp", bufs=4))
        cpool = top.enter_context(tc.tile_pool(name="cpool", bufs=6))
        # PSUM: 3 (proj) + 2 (scores) + 2 (transpose) + 1 (ctx) = 8 banks
        psP = top.enter_context(tc.tile_pool(name="psP", bufs=3, space="PSUM"))
        psb = top.enter_context(tc.tile_pool(name="psb", bufs=2, space="PSUM"))
        pst = top.enter_context(tc.tile_pool(name="pst", bufs=2, space="PSUM"))
        psc = top.enter_context(tc.tile_pool(name="psc", bufs=1, space="PSUM"))

        # warm the PE (HAM un-throttles after ~3.4us of activity) with
        # garbage matmuls on the identity tile while the real input
        # DMAs are still in flight
        warm = psP.tile([P, SC], F32, tag="proj", name="warm")
        for _ in range(32):
            nc.tensor.matmul(warm[:, :P], lhsT=ident[:],
                             rhs=ident[:], start=True, stop=True)

        def emit_out_proj(sc):
            # partial out-projection for chunk sc (emitted one round
            # late: its matmuls are always-ready PE work that absorbs
            # the attention phase's softmax latency)
            for s2 in range(SC // P):
                sb = sc * (SC // P) + s2
                for ec in range(D // SC):
                    psq = psP.tile([P, SC], F32, tag="proj", name="psq")
                    for h in range(NH):
                        nc.tensor.matmul(
                            psq[:],
                            lhsT=ctxT[h][:, sb * P:(sb + 1) * P],
                            rhs=wo_sb[:, h, ec * SC:(ec + 1) * SC],
                            start=(h == 0),
                            stop=(h == NH - 1),
                        )
                    po = cpool.tile([P, SC], CDT, tag="po", name="po")
                    if ec % 2 == 0:
                        nc.scalar.copy(po[:], psq[:])
                    else:
                        nc.vector.tensor_copy(po[:], psq[:])
                    nc.sync.dma_start(
                        pout_o.ap()[sb * P:(sb + 1) * P,
                                    ec * SC:(ec + 1) * SC], po[:])

        def load_x_chunk(x_r, sc):
            subs = []
            for i in range(NQ):
                t = xpool.tile([P, NDB // NQ, SC], CDT, tag=f"x{i}", name=f"x{i}")
                nc.sync.dma_start(
                    t[:], x_r[:, 4 * i:4 * i + 4, sc * SC:(sc + 1) * SC])
                subs.append(t)
            return subs

        def emit_proj_T(x_r, w_r, w_sb, b_sb, dstT, is_k, sc):
            # q/k projection for chunk sc (transposed layout)
            ssl = slice(sc * SC, (sc + 1) * SC)
            if sc == 0:
                # interleave weight-quarter / x-quarter transfers
                xs = []
                for i in range(NQ):
                    nc.sync.dma_start(
                        w_sb[i][:], w_r[:, 4 * i:4 * i + 4, :])
                    t = xpool.tile([P, NDB // NQ, SC], CDT,
                                   tag=f"x{i}", name=f"x{i}")
                    nc.sync.dma_start(
                        t[:], x_r[:, 4 * i:4 * i + 4, 0:SC])
                    xs.append(t)
            else:
                xs = load_x_chunk(x_r, sc)
            for hb in range(NH):
                ps = psP.tile([P, SC], F32, tag="proj", name="ps")
                for db in range(NDB):
                    nc.tensor.matmul(
                        ps[:],
                        lhsT=w_sb[db // 4][:, db % 4, hb * P:(hb + 1) * P],
                        rhs=xs[db // 4][:, db % 4, :],
                        start=(db == 0),
                        stop=(db == NDB - 1),
                    )
                nc.vector.tensor_scalar_add(
                    dstT[hb][:, ssl], ps[:], b_sb[:, hb:hb + 1])
                if is_k:
                    nc.sync.dma_start(kpT_r[hb][:, ssl], dstT[hb][:, ssl])

        # q-projection runs one round AHEAD of the attention phase so
        # every attention phase (incl. round 0) has a long stream of
        # always-ready projection matmuls behind it in the PE queue
        emit_proj_T(xq_r, wq_r, wq_sb, bq_sb, qpT, False, 0)

        for sc in range(NSC):
            ssl = slice(sc * SC, (sc + 1) * SC)

            emit_proj_T(xk_r, wk_r, wk_sb, bk_sb, kpT, True, sc)

            # ---- v projection for this chunk (natural layout) ----
            if sc == 0:
                nc.sync.dma_start(wv_sb[:], wv_r)
            xvs = load_x_chunk(xv_r, sc)
            if sc == 0:
                nc.sync.dma_start(wo_sb[:], wo_r)
            for s2 in range(SC // P):
                sb = sc * (SC // P) + s2
                ps2 = psP.tile([P, GD], F32, tag="proj", name="ps2")
                for db in range(NDB):
                    nc.tensor.matmul(
                        ps2[:],
                        lhsT=xvs[db // 4][:, db % 4, s2 * P:(s2 + 1) * P],
                        rhs=wv_sb[:, db, :],
                        start=(db == 0),
                        stop=(db == NDB - 1),
                    )
                nc.vector.tensor_tensor(vpB[sb][:], ps2[:], bvb_sb[:], AluOp.add)
                nc.sync.dma_start(vp_o.ap()[sb * P:(sb + 1) * P, :], vpB[sb][:])

            if sc > 0:
                emit_out_proj(sc - 1)
            if sc + 1 < NSC:
                emit_proj_T(xq_r, wq_r, wq_sb, bq_sb, qpT, False, sc + 1)

            # ---- windowed attention for this chunk's 4 query blocks ----
            qc = sc
            kjbase = 4 * qc - 2
            for h in range(NH):
                pb = pbufs[(qc * NH + h) % 4]
                probBs = []
                for t in range(4):
                    qb = qc * 4 + t
                    qs = qb * P
                    kj_lo = max(0, qs - WIN)
                    wdt = qs + P - kj_lo          # 128 / 256 / 384
                    moff = 3 * P - wdt
                    nblk = wdt // P
                    ps_s = psb.tile([P, 3 * P], F32, tag="ps_s", name="ps_s")
                    # scores, then the mask accumulated on the boundary
                    # key-blocks only (the middle block is all-allowed;
                    # the last block is always mask_sb[:, 2P:3P])
                    nc.tensor.matmul(
                        ps_s[:, :wdt],
                        lhsT=qpT[h][:, qs:qs + P],
                        rhs=kpT[h][:, kj_lo:kj_lo + wdt],
                        start=True,
                        stop=False,
                    )
                    if nblk == 3:
                        nc.tensor.matmul(
                            ps_s[:, :P],
                            lhsT=ident[:],
                            rhs=mask_sb[:, 0:P],
                            start=False,
                            stop=False,
                        )
                    nc.tensor.matmul(
                        ps_s[:, wdt - P:wdt],
                        lhsT=ident[:],
                        rhs=mask_sb[:, 2 * P:3 * P],
                        start=False,
                        stop=True,
                    )
                    exps = wkp.tile([P, 3 * P], CDT, tag="exps", name="exps")
                    rsum = wkp.tile([P, 1], F32, tag="rsum", name="rsum")
                    nc.scalar.activation(exps[:, :wdt], ps_s[:, :wdt],
                                         ActFn.Exp, scale=float(SCALE),
                                         accum_out=rsum[:])
                    rinv = wkp.tile([P, 1], F32, tag="rinv", name="rinv")
                    nc.vector.reciprocal(rinv[:], rsum[:])
                    probB = wkp.tile([P, 3 * P], CDT, tag="probB", name="probB")
                    nc.vector.tensor_scalar_mul(probB[:, :wdt],
                                                exps[:, :wdt], rinv[:])
                    probBs.append((probB, kj_lo))
                # transposes grouped per destination key-block: the
                # valid query blocks per rel are consecutive, so 1-3
                # transposes share one PSUM tile and drain with a
                # single wide copy (half the DVE copies, and the copy
                # overlaps the next group's transposes)
                for rel in range(2 if qc == 0 else 0, 6):
                    t0 = max(0, rel - 2)
                    t1 = min(3, rel)
                    ps_t = pst.tile([P, 3 * P], CDT, tag="ps_t", name="ps_t")
                    for t in range(t0, t1 + 1):
                        probB, kj_lo = probBs[t]
                        j = (kjbase + rel) - kj_lo // P
                        nc.tensor.transpose(
                            ps_t[:, (t - t0) * P:(t - t0 + 1) * P],
                            probB[:, j * P:(j + 1) * P],
                            ident[:])
                    nc.vector.tensor_copy(
                        pb[:, rel, t0 * P:(t1 + 1) * P],
                        ps_t[:, 0:(t1 - t0 + 1) * P])
                # ctx in two half-width groups over only the key blocks
                # that can be non-zero for that query pair (the padded
                # full-width version wastes ~1/3 of the PE cycles)
                ps_pv = psc.tile([P, SC], F32, tag="ps_pv", name="ps_pv")
                for pr in range(2):
                    cs = slice(pr * 2 * P, (pr + 1) * 2 * P)
                    rels = [r for r in range(2 * pr, 2 * pr + 4)
                            if kjbase + r >= 0]
                    for i2, r in enumerate(rels):
                        nc.tensor.matmul(
                            ps_pv[:, cs],
                            lhsT=vpB[kjbase + r][:, h * P:(h + 1) * P],
                            rhs=pb[:, r, cs],
                            start=(i2 == 0),
                            stop=(i2 == len(rels) - 1),
                        )
                nc.scalar.copy(ctxT[h][:, qc * SC:(qc + 1) * SC], ps_pv[:])

        emit_out_proj(NSC - 1)

    nc.compile()
    return nc


def _band_mask():
    i = np.arange(P)[:, None]
    j = np.arange(3 * P)[None, :]
    return np.where((j >= i) & (j <= i + WIN), 0.0, -1e6).astype(np.float32)


def kernel(q, k, v, Wq, bq, Wk, bk, Wv, bv, Wo, bo):
    global LAST_RESULTS
    q = np.asarray(q, np.float32)
    k = np.asarray(k, np.float32)
    v = np.asarray(v, np.float32)
    Wq = np.asarray(Wq, np.float32)
    Wk = np.asarray(Wk, np.float32)
    Wv = np.asarray(Wv, np.float32)
    Wo = np.asarray(Wo, np.float32)
    bq = np.asarray(bq, np.float32)
    bk = np.asarray(bk, np.float32)
    bv = np.asarray(bv, np.float32)
    bo = np.asarray(bo, np.float32)

    if "nc" not in _CACHE:
        _CACHE["nc"] = _build_nc()
    nc = _CACHE["nc"]
    from concourse.bass_utils import run_bass_kernel_spmd

    if _mm_dtype_name() == "bf16":
        import ml_dtypes

        cdt = ml_dtypes.bfloat16
    else:
        cdt = np.float32

    mask = _band_mask().astype(cdt)
    xT = {}
    for b in range(B):
        xT[("q", b)] = np.ascontiguousarray(q[b].T).astype(cdt)
        xT[("k", b)] = np.ascontiguousarray(k[b].T).astype(cdt)
        xT[("v", b)] = np.ascontiguousarray(v[b].T).astype(cdt)

    in_maps = []
    for core in range(8):
        b, g = divmod(core, 4)
        sl = slice(g * GD, (g + 1) * GD)
        in_maps.append({
            "xq_T": xT[("q", b)],
            "xk_T": xT[("k", b)],
            "xv_T": xT[("v", b)],
            "wq": np.ascontiguousarray(Wq[:, sl]).astype(cdt),
            "wk": np.ascontiguousarray(Wk[:, sl]).astype(cdt),
            "wv": np.ascontiguousarray(Wv[:, sl]).astype(cdt),
            "wo": np.ascontiguousarray(Wo[sl, :]).astype(cdt),
            "bq2": np.ascontiguousarray(bq[sl]),
            "bk2": np.ascontiguousarray(bk[sl]),
            "bvb": np.ascontiguousarray(np.broadcast_to(bv[sl], (P, GD))),
            "maskd": mask,
        })

    trace = os.environ.get("KERNEL_TRACE", "0") == "1"
    res = run_bass_kernel_spmd(nc, in_maps, core_ids=list(range(8)), trace=trace)
    LAST_RESULTS = res

    out = np.zeros((B, S, D), np.float64)
    kp = np.empty((B, S, D), np.float32)
    vp = np.empty((B, S, D), np.float32)
    for core in range(8):
        b, g = divmod(core, 4)
        sl = slice(g * GD, (g + 1) * GD)
        r = res.results[core]
        kp[b][:, sl] = r["kpT"].astype(np.float32).T
        vp[b][:, sl] = r["vp"].astype(np.float32)
        out[b] += r["pout"].astype(np.float64)
    out = (out + bo.astype(np.float64)).astype(np.float32)
    return out, kp, vp


# revision 58
# speedup vs baseline: 1.1769x; 1.1769x over previous
"""Trainium2 Bass kernel for a sparse (sliding-window) attention layer.

Reference computation (B=2, S=2048, D=2048, H=16 heads, window=256, fp32):
    qp = q @ Wq + bq ; kp = k @ Wk + bk ; vp = v @ Wv + bv
    per-head scores with mask (0 <= q_idx - k_idx <= 256), softmax, ctx
    out = merge_heads(ctx) @ Wo + bo
    returns (out, kp, vp)

Sharding: 8 cores = 2 (batch) x 4 (head groups of 4 heads / 512 dims).
Each core computes its batch's projections for its 512 output dims
(transposed layout for q/k so attention feeds straight into the PE),
the windowed attention for its 4 heads, and a partial out-projection
(rows of Wo owned by its heads).  Host sums the 4 partial outputs per
batch (the "out_proj all-reduce") and concatenates kp/vp slices.

The kernel is a 4-round pipeline over 512-token seq chunks: each round
projects q/k/v for the chunk, runs the windowed attention for the
chunk's 4 query blocks on all 4 heads, and emits the chunk's partial
out-projection.  This keeps the PE array streaming continuously (no
HAM re-throttle) and spreads DVE/ACT/DMA work evenly.

Engine budget choices:
  - the sliding-window mask is added on the TENSOR engine (an extra
    PSUM-accumulating matmul with an identity stationary) instead of a
    DVE tensor_tensor, keeping the Vector engine off the critical path
  - the padded prob buffers (transposed probs for the ctx matmul) are
    zeroed ONCE: the pad blocks are never written by any round
  - kp/vp/pout DRAM outputs are bf16 (host upconverts); matmuls are
    bf16 with fp32 PSUM accumulation; softmax statistics stay fp32
"""

import os
import sys

import numpy as np

B = 2
S = 2048
D = 2048
GD = 512          # dims per core (4 heads x 128)
NH = 4            # heads per core
P = 128
WIN = 256         # sliding window
NDB = D // P      # 16 contraction blocks
SC = 512          # seq chunk (one pipeline round)
NSC = S // SC     # 4 rounds
NSB = S // P      # 16 seq blocks
SCALE = 1.0 / np.sqrt(P)

_CACHE = {}
LAST_RESULTS = None


def _mm_dtype_name():
    return os.environ.get("KERNEL_MM_DT", "bf16")


def _build_nc():
    sys.path.insert(0, "/opt/trn_rl_repo")
    import concourse.bass as bass  # noqa: F401
    import concourse.tile as tile
    from concourse import mybir, bacc
    from concourse.masks import make_identity
    from contextlib import ExitStack

    F32 = mybir.dt.float32
    CDT = mybir.dt.bfloat16 if _mm_dtype_name() == "bf16" else F32

    nc = bacc.Bacc("TRN2", target_bir_lowering=False, debug=False, num_devices=8)

    xq_T = nc.dram_tensor("xq_T", [D, S], CDT, kind="ExternalInput")
    xk_T = nc.dram_tensor("xk_T", [D, S], CDT, kind="ExternalInput")
    xv_T = nc.dram_tensor("xv_T", [D, S], CDT, kind="ExternalInput")
    wq = nc.dram_tensor("wq", [D, GD], CDT, kind="ExternalInput")
    wk = nc.dram_tensor("wk", [D, GD], CDT, kind="ExternalInput")
    wv = nc.dram_tensor("wv", [D, GD], CDT, kind="ExternalInput")
    wo = nc.dram_tensor("wo", [GD, D], CDT, kind="ExternalInput")
    bq2 = nc.dram_tensor("bq2", [GD], F32, kind="ExternalInput")
    bk2 = nc.dram_tensor("bk2", [GD], F32, kind="ExternalInput")
    bvb = nc.dram_tensor("bvb", [P, GD], F32, kind="ExternalInput")
    maskd = nc.dram_tensor("maskd", [P, 3 * P], CDT, kind="ExternalInput")

    kpT_o = nc.dram_tensor("kpT", [GD, S], CDT, kind="ExternalOutput")
    vp_o = nc.dram_tensor("vp", [S, GD], CDT, kind="ExternalOutput")
    pout_o = nc.dram_tensor("pout", [S, D], CDT, kind="ExternalOutput")

    # partition-major views of the DRAM operands
    xq_r = xq_T.ap().rearrange("(do p) s -> p do s", p=P)
    xk_r = xk_T.ap().rearrange("(do p) s -> p do s", p=P)
    xv_r = xv_T.ap().rearrange("(do p) s -> p do s", p=P)
    wq_r = wq.ap().rearrange("(do p) n -> p do n", p=P)
    wk_r = wk.ap().rearrange("(do p) n -> p do n", p=P)
    wv_r = wv.ap().rearrange("(do p) n -> p do n", p=P)
    wo_r = wo.ap().rearrange("(h p) n -> p h n", p=P)
    bq_r = bq2.ap().rearrange("(h p) -> p h", p=P)
    bk_r = bk2.ap().rearrange("(h p) -> p h", p=P)
    kpT_r = kpT_o.ap().rearrange("(h p) s -> h p s", p=P)

    AluOp = mybir.AluOpType
    ActFn = mybir.ActivationFunctionType

    NQ = 4            # weight/x sub-tiles (4 db blocks each)

    with tile.TileContext(nc) as tc, ExitStack() as top:
        const = top.enter_context(tc.tile_pool(name="const", bufs=1))
        junk = const.tile([P, SC], CDT, name="junk")
        nc.gpsimd.memset(junk[:], 0.125)
        ident = const.tile([P, P], CDT, name="ident")
        make_identity(nc, ident[:])
        mask_sb = const.tile([P, 3 * P], CDT, name="mask_sb")
        nc.sync.dma_start(mask_sb[:], maskd.ap())
        bq_sb = const.tile([P, NH], F32, name="bq_sb")
        nc.sync.dma_start(bq_sb[:], bq_r)
        bk_sb = const.tile([P, NH], F32, name="bk_sb")
        nc.sync.dma_start(bk_sb[:], bk_r)
        bvb_sb = const.tile([P, GD], F32, name="bvb_sb")
        nc.sync.dma_start(bvb_sb[:], bvb.ap())

        # weights: q/k split into 4 sub-tiles so the first matmuls can
        # start as soon as the first quarter + first x quarter land.
        # DMAs are emitted lazily (just before first use) so the DMA
        # queues service the startup-critical transfers first.
        wpool = top.enter_context(tc.tile_pool(name="wpool", bufs=1))
        wq_sb = [wpool.tile([P, NDB // NQ, GD], CDT, name=f"wq_sb{i}")
                 for i in range(NQ)]
        wk_sb = [wpool.tile([P, NDB // NQ, GD], CDT, name=f"wk_sb{i}")
                 for i in range(NQ)]
        wv_sb = wpool.tile([P, NDB, GD], CDT, name="wv_sb")
        wo_sb = wpool.tile([P, NH, D], CDT, name="wo_sb")

        # long-lived activations
        persist1 = top.enter_context(tc.tile_pool(name="persist1", bufs=1))
        qpT = [persist1.tile([P, S], CDT, name=f"qpT{h}") for h in range(NH)]
        kpT = [persist1.tile([P, S], CDT, name=f"kpT{h}") for h in range(NH)]
        persist2 = top.enter_context(tc.tile_pool(name="persist2", bufs=1))
        vpB = [persist2.tile([P, GD], CDT, name=f"vpB{sb}") for sb in range(NSB)]
        persist3 = top.enter_context(tc.tile_pool(name="persist3", bufs=1))
        ctxT = [persist3.tile([P, S], CDT, name=f"ctxT{h}") for h in range(NH)]

        # transposed-prob buffers: [key-block rel 0..5, query 0..511].
        # Pad blocks (rel<t or rel>t+2) are never written by any round,
        # so a single memset keeps them zero for the whole kernel.
        persist4 = top.enter_context(tc.tile_pool(name="persist4", bufs=1))
        pbufs = [persist4.tile([P, 6, SC], CDT, name=f"pbuf{i}") for i in range(4)]
        for pb in pbufs:
            nc.gpsimd.memset(pb[:], 0.0)

        # working pools
        xpool = top.enter_context(tc.tile_pool(name="xpool", bufs=2))
        wkp = top.enter_context(tc.tile_pool(name="wkp", bufs=4))
        cpool = top.enter_context(tc.tile_pool(name="cpool", bufs=6))
        # PSUM: 3 (proj) + 2 (scores) + 2 (transpose) + 1 (ctx) = 8 banks
        psP = top.enter_context(tc.tile_pool(name="psP", bufs=3, space="PSUM"))
        psb = top.enter_context(tc.tile_pool(name="psb", bufs=2, space="PSUM"))
        pst = top.enter_context(tc.tile_pool(name="pst", bufs=2, space="PSUM"))
        psc = top.enter_context(tc.tile_pool(name="psc", bufs=1, space="PSUM"))

        # warm the PE (HAM un-throttles after ~3.4us of activity) with
        # garbage matmuls on the identity tile while the real input
        # DMAs are still in flight
        warm = psP.tile([P, SC], F32, tag="proj", name="warm")
        for _ in range(32):
            nc.tensor.matmul(warm[:, :P], lhsT=ident[:],
                             rhs=ident[:], start=True, stop=True)

        def emit_out_proj(sc):
            # partial out-projection for chunk sc (emitted one round
            # late: its matmuls are always-ready PE work that absorbs
            # the attention phase's softmax latency)
            for s2 in range(SC // P):
                sb = sc * (SC // P) + s2
                for ec in range(D // SC):
                    psq = psP.tile([P, SC], F32, tag="proj", name="psq")
                    for h in range(NH):
                        nc.tensor.matmul(
                            psq[:],
                            lhsT=ctxT[h][:, sb * P:(sb + 1) * P],
                            rhs=wo_sb[:, h, ec * SC:(ec + 1) * SC],
                            start=(h == 0),
                            stop=(h == NH - 1),
                        )
                    po = cpool.tile([P, SC], CDT, tag="po", name="po")
                    if ec % 2 == 0:
                        nc.scalar.copy(po[:], psq[:])
                    else:
                        nc.vector.tensor_copy(po[:], psq[:])
                    nc.sync.dma_start(
                        pout_o.ap()[sb * P:(sb + 1) * P,
                                    ec * SC:(ec + 1) * SC], po[:])

        def load_x_chunk(x_r, sc):
            subs = []
            for i in range(NQ):
                t = xpool.tile([P, NDB // NQ, SC], CDT, tag=f"x{i}", name=f"x{i}")
                nc.sync.dma_start(
                    t[:], x_r[:, 4 * i:4 * i + 4, sc * SC:(sc + 1) * SC])
                subs.append(t)
            return subs

        for sc in range(NSC):
            ssl = slice(sc * SC, (sc + 1) * SC)

            # ---- q / k projections for this chunk (transposed layout) ----
            for x_r, w_r, w_sb, b_sb, dstT, is_k in (
                (xq_r, wq_r, wq_sb, bq_sb, qpT, False),
                (xk_r, wk_r, wk_sb, bk_sb, kpT, True),
            ):
                if sc == 0:
                    # interleave weight-quarter / x-quarter transfers
                    xs = []
                    for i in range(NQ):
                        nc.sync.dma_start(
                            w_sb[i][:], w_r[:, 4 * i:4 * i + 4, :])
                        t = xpool.tile([P, NDB // NQ, SC], CDT,
                                       tag=f"x{i}", name=f"x{i}")
                        nc.sync.dma_start(
                            t[:], x_r[:, 4 * i:4 * i + 4, 0:SC])
                        xs.append(t)
                else:
                    xs = load_x_chunk(x_r, sc)
                for hb in range(NH):
                    ps = psP.tile([P, SC], F32, tag="proj", name="ps")
                    for db in range(NDB):
                        nc.tensor.matmul(
                            ps[:],
                            lhsT=w_sb[db // 4][:, db % 4, hb * P:(hb + 1) * P],
                            rhs=xs[db // 4][:, db % 4, :],
                            start=(db == 0),
                            stop=(db == NDB - 1),
                        )
                    nc.vector.tensor_scalar_add(
                        dstT[hb][:, ssl], ps[:], b_sb[:, hb:hb + 1])
                    if is_k:
                        nc.sync.dma_start(kpT_r[hb][:, ssl], dstT[hb][:, ssl])

            # ---- v projection for this chunk (natural layout) ----
            if sc == 0:
                nc.sync.dma_start(wv_sb[:], wv_r)
            xvs = load_x_chunk(xv_r, sc)
            if sc == 0:
                nc.sync.dma_start(wo_sb[:], wo_r)
            for s2 in range(SC // P):
                sb = sc * (SC // P) + s2
                ps2 = psP.tile([P, GD], F32, tag="proj", name="ps2")
                for db in range(NDB):
                    nc.tensor.matmul(
                        ps2[:],
                        lhsT=xvs[db // 4][:, db % 4, s2 * P:(s2 + 1) * P],
                        rhs=wv_sb[:, db, :],
                        start=(db == 0),
                        stop=(db == NDB - 1),
                    )
                nc.vector.tensor_tensor(vpB[sb][:], ps2[:], bvb_sb[:], AluOp.add)
                nc.sync.dma_start(vp_o.ap()[sb * P:(sb + 1) * P, :], vpB[sb][:])

            if sc > 0:
                emit_out_proj(sc - 1)

            # ---- windowed attention for this chunk's 4 query blocks ----
            qc = sc
            kjbase = 4 * qc - 2
            for h in range(NH):
                pb = pbufs[(qc * NH + h) % 4]
                probBs = []
                for t in range(4):
                    qb = qc * 4 + t
                    qs = qb * P
                    kj_lo = max(0, qs - WIN)
                    wdt = qs + P - kj_lo          # 128 / 256 / 384
                    moff = 3 * P - wdt
                    nblk = wdt // P
                    ps_s = psb.tile([P, 3 * P], F32, tag="ps_s", name="ps_s")
                    # scores, then the mask accumulated on the boundary
                    # key-blocks only (the middle block is all-allowed;
                    # the last block is always mask_sb[:, 2P:3P])
                    nc.tensor.matmul(
                        ps_s[:, :wdt],
                        lhsT=qpT[h][:, qs:qs + P],
                        rhs=kpT[h][:, kj_lo:kj_lo + wdt],
                        start=True,
                        stop=False,
                    )
                    if nblk == 3:
                        nc.tensor.matmul(
                            ps_s[:, :P],
                            lhsT=ident[:],
                            rhs=mask_sb[:, 0:P],
                            start=False,
                            stop=False,
                        )
                    nc.tensor.matmul(
                        ps_s[:, wdt - P:wdt],
                        lhsT=ident[:],
                        rhs=mask_sb[:, 2 * P:3 * P],
                        start=False,
                        stop=True,
                    )
                    exps = wkp.tile([P, 3 * P], CDT, tag="exps", name="exps")
                    rsum = wkp.tile([P, 1], F32, tag="rsum", name="rsum")
                    nc.scalar.activation(exps[:, :wdt], ps_s[:, :wdt],
                                         ActFn.Exp, scale=float(SCALE),
                                         accum_out=rsum[:])
                    rinv = wkp.tile([P, 1], F32, tag="rinv", name="rinv")
                    nc.vector.reciprocal(rinv[:], rsum[:])
                    probB = wkp.tile([P, 3 * P], CDT, tag="probB", name="probB")
                    nc.vector.tensor_scalar_mul(probB[:, :wdt],
                                                exps[:, :wdt], rinv[:])
                    probBs.append((probB, kj_lo))
                # transposes grouped per destination key-block: the
                # valid query blocks per rel are consecutive, so 1-3
                # transposes share one PSUM tile and drain with a
                # single wide copy (half the DVE copies, and the copy
                # overlaps the next group's transposes)
                for rel in range(2 if qc == 0 else 0, 6):
                    t0 = max(0, rel - 2)
                    t1 = min(3, rel)
                    ps_t = pst.tile([P, 3 * P], CDT, tag="ps_t", name="ps_t")
                    for t in range(t0, t1 + 1):
                        probB, kj_lo = probBs[t]
                        j = (kjbase + rel) - kj_lo // P
                        nc.tensor.transpose(
                            ps_t[:, (t - t0) * P:(t - t0 + 1) * P],
                            probB[:, j * P:(j + 1) * P],
                            ident[:])
                    nc.vector.tensor_copy(
                        pb[:, rel, t0 * P:(t1 + 1) * P],
                        ps_t[:, 0:(t1 - t0 + 1) * P])
                # ctx in two half-width groups over only the key blocks
                # that can be non-zero for that query pair (the padded
                # full-width version wastes ~1/3 of the PE cycles)
                ps_pv = psc.tile([P, SC], F32, tag="ps_pv", name="ps_pv")
                for pr in range(2):
                    cs = slice(pr * 2 * P, (pr + 1) * 2 * P)
                    rels = [r for r in range(2 * pr, 2 * pr + 4)
                            if kjbase + r >= 0]
                    for i2, r in enumerate(rels):
                        nc.tensor.matmul(
                            ps_pv[:, cs],
                            lhsT=vpB[kjbase + r][:, h * P:(h + 1) * P],
                            rhs=pb[:, r, cs],
                            start=(i2 == 0),
                            stop=(i2 == len(rels) - 1),
                        )
                nc.scalar.copy(ctxT[h][:, qc * SC:(qc + 1) * SC], ps_pv[:])

        emit_out_proj(NSC - 1)

    nc.compile()
    return nc


def _band_mask():
    i = np.arange(P)[:, None]
    j = np.arange(3 * P)[None, :]
    return np.where((j >= i) & (j <= i + WIN), 0.0, -1e6).astype(np.float32)


def kernel(q, k, v, Wq, bq, Wk, bk, Wv, bv, Wo, bo):
    global LAST_RESULTS
    q = np.asarray(q, np.float32)
    k = np.asarray(k, np.float32)
    v = np.asarray(v, np.float32)
    Wq = np.asarray(Wq, np.float32)
    Wk = np.asarray(Wk, np.float32)
    Wv = np.asarray(Wv, np.float32)
    Wo = np.asarray(Wo, np.float32)
    bq = np.asarray(bq, np.float32)
    bk = np.asarray(bk, np.float32)
    bv = np.asarray(bv, np.float32)
    bo = np.asarray(bo, np.float32)

    if "nc" not in _CACHE:
        _CACHE["nc"] = _build_nc()
    nc = _CACHE["nc"]
    from concourse.bass_utils import run_bass_kernel_spmd

    if _mm_dtype_name() == "bf16":
        import ml_dtypes

        cdt = ml_dtypes.bfloat16
    else:
        cdt = np.float32

    mask = _band_mask().astype(cdt)
    xT = {}
    for b in range(B):
        xT[("q", b)] = np.ascontiguousarray(q[b].T).astype(cdt)
        xT[("k", b)] = np.ascontiguousarray(k[b].T).astype(cdt)
        xT[("v", b)] = np.ascontiguousarray(v[b].T).astype(cdt)

    in_maps = []
    for core in range(8):
        b, g = divmod(core, 4)
        sl = slice(g * GD, (g + 1) * GD)
        in_maps.append({
            "xq_T": xT[("q", b)],
            "xk_T": xT[("k", b)],
            "xv_T": xT[("v", b)],
            "wq": np.ascontiguousarray(Wq[:, sl]).astype(cdt),
            "wk": np.ascontiguousarray(Wk[:, sl]).astype(cdt),
            "wv": np.ascontiguousarray(Wv[:, sl]).astype(cdt),
            "wo": np.ascontiguousarray(Wo[sl, :]).astype(cdt),
            "bq2": np.ascontiguousarray(bq[sl]),
            "bk2": np.ascontiguousarray(bk[sl]),
            "bvb": np.ascontiguousarray(np.broadcast_to(bv[sl], (P, GD))),
            "maskd": mask,
        })

    trace = os.environ.get("KERNEL_TRACE", "0") == "1"
    res = run_bass_kernel_spmd(nc, in_maps, core_ids=list(range(8)), trace=trace)
    LAST_RESULTS = res

    out = np.zeros((B, S, D), np.float64)
    kp = np.empty((B, S, D), np.float32)
    vp = np.empty((B, S, D), np.float32)
    for core in range(8):
        b, g = divmod(core, 4)
        sl = slice(g * GD, (g + 1) * GD)
        r = res.results[core]
        kp[b][:, sl] = r["kpT"].astype(np.float32).T
        vp[b][:, sl] = r["vp"].astype(np.float32)
        out[b] += r["pout"].astype(np.float64)
    out = (out + bo.astype(np.float64)).astype(np.float32)
    return out, kp, vp


# revision 61
# speedup vs baseline: 1.1892x; 1.0104x over previous
"""Trainium2 Bass kernel for a sparse (sliding-window) attention layer.

Reference computation (B=2, S=2048, D=2048, H=16 heads, window=256, fp32):
    qp = q @ Wq + bq ; kp = k @ Wk + bk ; vp = v @ Wv + bv
    per-head scores with mask (0 <= q_idx - k_idx <= 256), softmax, ctx
    out = merge_heads(ctx) @ Wo + bo
    returns (out, kp, vp)

Sharding: 8 cores = 2 (batch) x 4 (head groups of 4 heads / 512 dims).
Each core computes its batch's projections for its 512 output dims
(transposed layout for q/k so attention feeds straight into the PE),
the windowed attention for its 4 heads, and a partial out-projection
(rows of Wo owned by its heads).  Host sums the 4 partial outputs per
batch (the "out_proj all-reduce") and concatenates kp/vp slices.

The kernel is a 4-round pipeline over 512-token seq chunks: each round
projects q/k/v for the chunk, runs the windowed attention for the
chunk's 4 query blocks on all 4 heads, and emits the chunk's partial
out-projection.  This keeps the PE array streaming continuously (no
HAM re-throttle) and spreads DVE/ACT/DMA work evenly.

Engine budget choices:
  - the sliding-window mask is added on the TENSOR engine (an extra
    PSUM-accumulating matmul with an identity stationary) instead of a
    DVE tensor_tensor, keeping the Vector engine off the critical path
  - the padded prob buffers (transposed probs for the ctx matmul) are
    zeroed ONCE: the pad blocks are never written by any round
  - kp/vp/pout DRAM outputs are bf16 (host upconverts); matmuls are
    bf16 with fp32 PSUM accumulation; softmax statistics stay fp32
"""

import os
import sys

import numpy as np

B = 2
S = 2048
D = 2048
GD = 512          # dims per core (4 heads x 128)
NH = 4            # heads per core
P = 128
WIN = 256         # sliding window
NDB = D // P      # 16 contraction blocks
SC = 512          # seq chunk (one pipeline round)
NSC = S // SC     # 4 rounds
NSB = S // P      # 16 seq blocks
SCALE = 1.0 / np.sqrt(P)

_CACHE = {}
LAST_RESULTS = None


def _mm_dtype_name():
    return os.environ.get("KERNEL_MM_DT", "bf16")


def _build_nc():
    sys.path.insert(0, "/opt/trn_rl_repo")
    import concourse.bass as bass  # noqa: F401
    import concourse.tile as tile
    from concourse import mybir, bacc
    from concourse.masks import make_identity
    from contextlib import ExitStack

    F32 = mybir.dt.float32
    CDT = mybir.dt.bfloat16 if _mm_dtype_name() == "bf16" else F32

    nc = bacc.Bacc("TRN2", target_bir_lowering=False, debug=False, num_devices=8)

    xq_T = nc.dram_tensor("xq_T", [D, S], CDT, kind="ExternalInput")
    xk_T = nc.dram_tensor("xk_T", [D, S], CDT, kind="ExternalInput")
    xv_T = nc.dram_tensor("xv_T", [D, S], CDT, kind="ExternalInput")
    wq = nc.dram_tensor("wq", [D, GD], CDT, kind="ExternalInput")
    wk = nc.dram_tensor("wk", [D, GD], CDT, kind="ExternalInput")
    wv = nc.dram_tensor("wv", [D, GD], CDT, kind="ExternalInput")
    wo = nc.dram_tensor("wo", [GD, D], CDT, kind="ExternalInput")
    bq2 = nc.dram_tensor("bq2", [GD], F32, kind="ExternalInput")
    bk2 = nc.dram_tensor("bk2", [GD], F32, kind="ExternalInput")
    bvb = nc.dram_tensor("bvb", [P, GD], F32, kind="ExternalInput")
    maskd = nc.dram_tensor("maskd", [P, 3 * P], CDT, kind="ExternalInput")

    kpT_o = nc.dram_tensor("kpT", [GD, S], CDT, kind="ExternalOutput")
    vp_o = nc.dram_tensor("vp", [S, GD], CDT, kind="ExternalOutput")
    pout_o = nc.dram_tensor("pout", [S, D], CDT, kind="ExternalOutput")

    # partition-major views of the DRAM operands
    xq_r = xq_T.ap().rearrange("(do p) s -> p do s", p=P)
    xk_r = xk_T.ap().rearrange("(do p) s -> p do s", p=P)
    xv_r = xv_T.ap().rearrange("(do p) s -> p do s", p=P)
    wq_r = wq.ap().rearrange("(do p) n -> p do n", p=P)
    wk_r = wk.ap().rearrange("(do p) n -> p do n", p=P)
    wv_r = wv.ap().rearrange("(do p) n -> p do n", p=P)
    wo_r = wo.ap().rearrange("(h p) n -> p h n", p=P)
    bq_r = bq2.ap().rearrange("(h p) -> p h", p=P)
    bk_r = bk2.ap().rearrange("(h p) -> p h", p=P)
    kpT_r = kpT_o.ap().rearrange("(h p) s -> h p s", p=P)

    AluOp = mybir.AluOpType
    ActFn = mybir.ActivationFunctionType

    NQ = 4            # weight/x sub-tiles (4 db blocks each)

    with tile.TileContext(nc) as tc, ExitStack() as top:
        const = top.enter_context(tc.tile_pool(name="const", bufs=1))
        junk = const.tile([P, SC], CDT, name="junk")
        nc.gpsimd.memset(junk[:], 0.125)
        ident = const.tile([P, P], CDT, name="ident")
        make_identity(nc, ident[:])
        mask_sb = const.tile([P, 3 * P], CDT, name="mask_sb")
        nc.sync.dma_start(mask_sb[:], maskd.ap())
        bq_sb = const.tile([P, NH], F32, name="bq_sb")
        nc.sync.dma_start(bq_sb[:], bq_r)
        bk_sb = const.tile([P, NH], F32, name="bk_sb")
        nc.sync.dma_start(bk_sb[:], bk_r)
        bvb_sb = const.tile([P, GD], F32, name="bvb_sb")
        nc.sync.dma_start(bvb_sb[:], bvb.ap())

        # weights: q/k split into 4 sub-tiles so the first matmuls can
        # start as soon as the first quarter + first x quarter land.
        # DMAs are emitted lazily (just before first use) so the DMA
        # queues service the startup-critical transfers first.
        wpool = top.enter_context(tc.tile_pool(name="wpool", bufs=1))
        wq_sb = [wpool.tile([P, NDB // NQ, GD], CDT, name=f"wq_sb{i}")
                 for i in range(NQ)]
        wk_sb = [wpool.tile([P, NDB // NQ, GD], CDT, name=f"wk_sb{i}")
                 for i in range(NQ)]
        wv_sb = wpool.tile([P, NDB, GD], CDT, name="wv_sb")
        wo_sb = wpool.tile([P, NH, D], CDT, name="wo_sb")

        # long-lived activations
        persist1 = top.enter_context(tc.tile_pool(name="persist1", bufs=1))
        qpT = [persist1.tile([P, S], CDT, name=f"qpT{h}") for h in range(NH)]
        kpT = [persist1.tile([P, S], CDT, name=f"kpT{h}") for h in range(NH)]
        persist2 = top.enter_context(tc.tile_pool(name="persist2", bufs=1))
        vpB = [persist2.tile([P, GD], CDT, name=f"vpB{sb}") for sb in range(NSB)]
        persist3 = top.enter_context(tc.tile_pool(name="persist3", bufs=1))
        ctxT = [persist3.tile([P, S], CDT, name=f"ctxT{h}") for h in range(NH)]

        # transposed-prob buffers: [key-block rel 0..5, query 0..511].
        # Pad blocks (rel<t or rel>t+2) are never written by any round,
        # so a single memset keeps them zero for the whole kernel.
        persist4 = top.enter_context(tc.tile_pool(name="persist4", bufs=1))
        pbufs = [persist4.tile([P, 6, SC], CDT, name=f"pbuf{i}") for i in range(2)]
        for pb in pbufs:
            nc.gpsimd.memset(pb[:], 0.0)

        # working pools
        xpool = top.enter_context(tc.tile_pool(name="xpool", bufs=2))
        wkp = top.enter_context(tc.tile_pool(name="wkp", bufs=8))
        cpool = top.enter_context(tc.tile_pool(name="cpool", bufs=6))
        # PSUM: 3 (proj) + 2 (scores) + 2 (transpose) + 1 (ctx) = 8 banks
        psP = top.enter_context(tc.tile_pool(name="psP", bufs=3, space="PSUM"))
        psb = top.enter_context(tc.tile_pool(name="psb", bufs=2, space="PSUM"))
        pst = top.enter_context(tc.tile_pool(name="pst", bufs=2, space="PSUM"))
        psc = top.enter_context(tc.tile_pool(name="psc", bufs=1, space="PSUM"))

        # warm the PE (HAM un-throttles after ~3.4us of activity) with
        # garbage matmuls on the identity tile while the real input
        # DMAs are still in flight
        warm = psP.tile([P, SC], F32, tag="proj", name="warm")
        for _ in range(32):
            nc.tensor.matmul(warm[:, :P], lhsT=ident[:],
                             rhs=ident[:], start=True, stop=True)

        def emit_out_proj(sc):
            # partial out-projection for chunk sc (emitted one round
            # late: its matmuls are always-ready PE work that absorbs
            # the attention phase's softmax latency)
            for s2 in range(SC // P):
                sb = sc * (SC // P) + s2
                for ec in range(D // SC):
                    psq = psP.tile([P, SC], F32, tag="proj", name="psq")
                    for h in range(NH):
                        nc.tensor.matmul(
                            psq[:],
                            lhsT=ctxT[h][:, sb * P:(sb + 1) * P],
                            rhs=wo_sb[:, h, ec * SC:(ec + 1) * SC],
                            start=(h == 0),
                            stop=(h == NH - 1),
                        )
                    po = cpool.tile([P, SC], CDT, tag="po", name="po")
                    if ec % 2 == 0:
                        nc.scalar.copy(po[:], psq[:])
                    else:
                        nc.vector.tensor_copy(po[:], psq[:])
                    nc.sync.dma_start(
                        pout_o.ap()[sb * P:(sb + 1) * P,
                                    ec * SC:(ec + 1) * SC], po[:])

        def load_x_chunk(x_r, sc):
            subs = []
            for i in range(NQ):
                t = xpool.tile([P, NDB // NQ, SC], CDT, tag=f"x{i}", name=f"x{i}")
                nc.sync.dma_start(
                    t[:], x_r[:, 4 * i:4 * i + 4, sc * SC:(sc + 1) * SC])
                subs.append(t)
            return subs

        for sc in range(NSC):
            ssl = slice(sc * SC, (sc + 1) * SC)

            # ---- q / k projections for this chunk (transposed layout) ----
            for x_r, w_r, w_sb, b_sb, dstT, is_k in (
                (xq_r, wq_r, wq_sb, bq_sb, qpT, False),
                (xk_r, wk_r, wk_sb, bk_sb, kpT, True),
            ):
                if sc == 0:
                    # interleave weight-quarter / x-quarter transfers
                    xs = []
                    for i in range(NQ):
                        nc.sync.dma_start(
                            w_sb[i][:], w_r[:, 4 * i:4 * i + 4, :])
                        t = xpool.tile([P, NDB // NQ, SC], CDT,
                                       tag=f"x{i}", name=f"x{i}")
                        nc.sync.dma_start(
                            t[:], x_r[:, 4 * i:4 * i + 4, 0:SC])
                        xs.append(t)
                else:
                    xs = load_x_chunk(x_r, sc)
                for hb in range(NH):
                    ps = psP.tile([P, SC], F32, tag="proj", name="ps")
                    for db in range(NDB):
                        nc.tensor.matmul(
                            ps[:],
                            lhsT=w_sb[db // 4][:, db % 4, hb * P:(hb + 1) * P],
                            rhs=xs[db // 4][:, db % 4, :],
                            start=(db == 0),
                            stop=(db == NDB - 1),
                        )
                    nc.vector.tensor_scalar_add(
                        dstT[hb][:, ssl], ps[:], b_sb[:, hb:hb + 1])
                    if is_k:
                        nc.sync.dma_start(kpT_r[hb][:, ssl], dstT[hb][:, ssl])

            # ---- v projection for this chunk (natural layout) ----
            if sc == 0:
                nc.sync.dma_start(wv_sb[:], wv_r)
            xvs = load_x_chunk(xv_r, sc)
            if sc == 0:
                nc.sync.dma_start(wo_sb[:], wo_r)
            for s2 in range(SC // P):
                sb = sc * (SC // P) + s2
                ps2 = psP.tile([P, GD], F32, tag="proj", name="ps2")
                for db in range(NDB):
                    nc.tensor.matmul(
                        ps2[:],
                        lhsT=xvs[db // 4][:, db % 4, s2 * P:(s2 + 1) * P],
                        rhs=wv_sb[:, db, :],
                        start=(db == 0),
                        stop=(db == NDB - 1),
                    )
                nc.vector.tensor_tensor(vpB[sb][:], ps2[:], bvb_sb[:], AluOp.add)
                nc.sync.dma_start(vp_o.ap()[sb * P:(sb + 1) * P, :], vpB[sb][:])

            if sc > 0:
                emit_out_proj(sc - 1)

            # ---- windowed attention for this chunk's 4 query blocks ----
            qc = sc
            kjbase = 4 * qc - 2
            def emit_transp_ctx(h, pb, probBs):
                # transposes grouped per destination key-block, then
                # the ctx matmuls; called one head LATE so head h+1's
                # always-ready score matmuls sit ahead of these
                # softmax-gated instructions in the PE queue
                for rel in range(2 if qc == 0 else 0, 6):
                    t0 = max(0, rel - 2)
                    t1 = min(3, rel)
                    ps_t = pst.tile([P, 3 * P], CDT, tag="ps_t", name="ps_t")
                    for t in range(t0, t1 + 1):
                        probB, kj_lo = probBs[t]
                        j = (kjbase + rel) - kj_lo // P
                        nc.tensor.transpose(
                            ps_t[:, (t - t0) * P:(t - t0 + 1) * P],
                            probB[:, j * P:(j + 1) * P],
                            ident[:])
                    nc.vector.tensor_copy(
                        pb[:, rel, t0 * P:(t1 + 1) * P],
                        ps_t[:, 0:(t1 - t0 + 1) * P])
                ps_pv = psc.tile([P, SC], F32, tag="ps_pv", name="ps_pv")
                for pr in range(2):
                    cs = slice(pr * 2 * P, (pr + 1) * 2 * P)
                    rels = [r for r in range(2 * pr, 2 * pr + 4)
                            if kjbase + r >= 0]
                    for i2, r in enumerate(rels):
                        nc.tensor.matmul(
                            ps_pv[:, cs],
                            lhsT=vpB[kjbase + r][:, h * P:(h + 1) * P],
                            rhs=pb[:, r, cs],
                            start=(i2 == 0),
                            stop=(i2 == len(rels) - 1),
                        )
                nc.scalar.copy(ctxT[h][:, qc * SC:(qc + 1) * SC], ps_pv[:])

            prev = None
            for h in range(NH):
                pb = pbufs[(qc * NH + h) % 2]
                probBs = []
                for t in range(4):
                    qb = qc * 4 + t
                    qs = qb * P
                    kj_lo = max(0, qs - WIN)
                    wdt = qs + P - kj_lo          # 128 / 256 / 384
                    moff = 3 * P - wdt
                    nblk = wdt // P
                    ps_s = psb.tile([P, 3 * P], F32, tag="ps_s", name="ps_s")
                    # scores, then the mask accumulated on the boundary
                    # key-blocks only (the middle block is all-allowed;
                    # the last block is always mask_sb[:, 2P:3P])
                    nc.tensor.matmul(
                        ps_s[:, :wdt],
                        lhsT=qpT[h][:, qs:qs + P],
                        rhs=kpT[h][:, kj_lo:kj_lo + wdt],
                        start=True,
                        stop=False,
                    )
                    if nblk == 3:
                        nc.tensor.matmul(
                            ps_s[:, :P],
                            lhsT=ident[:],
                            rhs=mask_sb[:, 0:P],
                            start=False,
                            stop=False,
                        )
                    nc.tensor.matmul(
                        ps_s[:, wdt - P:wdt],
                        lhsT=ident[:],
                        rhs=mask_sb[:, 2 * P:3 * P],
                        start=False,
                        stop=True,
                    )
                    exps = wkp.tile([P, 3 * P], CDT, tag="exps", name="exps")
                    rsum = wkp.tile([P, 1], F32, tag="rsum", name="rsum")
                    nc.scalar.activation(exps[:, :wdt], ps_s[:, :wdt],
                                         ActFn.Exp, scale=float(SCALE),
                                         accum_out=rsum[:])
                    rinv = wkp.tile([P, 1], F32, tag="rinv", name="rinv")
                    nc.vector.reciprocal(rinv[:], rsum[:])
                    probB = wkp.tile([P, 3 * P], CDT, tag="probB", name="probB")
                    nc.vector.tensor_scalar_mul(probB[:, :wdt],
                                                exps[:, :wdt], rinv[:])
                    probBs.append((probB, kj_lo))
                if prev is not None:
                    emit_transp_ctx(*prev)
                prev = (h, pb, probBs)
            emit_transp_ctx(*prev)

        emit_out_proj(NSC - 1)

    nc.compile()
    return nc


def _band_mask():
    i = np.arange(P)[:, None]
    j = np.arange(3 * P)[None, :]
    return np.where((j >= i) & (j <= i + WIN), 0.0, -1e6).astype(np.float32)


def kernel(q, k, v, Wq, bq, Wk, bk, Wv, bv, Wo, bo):
    global LAST_RESULTS
    q = np.asarray(q, np.float32)
    k = np.asarray(k, np.float32)
    v = np.asarray(v, np.float32)
    Wq = np.asarray(Wq, np.float32)
    Wk = np.asarray(Wk, np.float32)
    Wv = np.asarray(Wv, np.float32)
    Wo = np.asarray(Wo, np.float32)
    bq = np.asarray(bq, np.float32)
    bk = np.asarray(bk, np.float32)
    bv = np.asarray(bv, np.float32)
    bo = np.asarray(bo, np.float32)

    if "nc" not in _CACHE:
        _CACHE["nc"] = _build_nc()
    nc = _CACHE["nc"]
    from concourse.bass_utils import run_bass_kernel_spmd

    if _mm_dtype_name() == "bf16":
        import ml_dtypes

        cdt = ml_dtypes.bfloat16
    else:
        cdt = np.float32

    mask = _band_mask().astype(cdt)
    xT = {}
    for b in range(B):
        xT[("q", b)] = np.ascontiguousarray(q[b].T).astype(cdt)
        xT[("k", b)] = np.ascontiguousarray(k[b].T).astype(cdt)
        xT[("v", b)] = np.ascontiguousarray(v[b].T).astype(cdt)

    in_maps = []
    for core in range(8):
        b, g = divmod(core, 4)
        sl = slice(g * GD, (g + 1) * GD)
        in_maps.append({
            "xq_T": xT[("q", b)],
            "xk_T": xT[("k", b)],
            "xv_T": xT[("v", b)],
            "wq": np.ascontiguousarray(Wq[:, sl]).astype(cdt),
            "wk": np.ascontiguousarray(Wk[:, sl]).astype(cdt),
            "wv": np.ascontiguousarray(Wv[:, sl]).astype(cdt),
            "wo": np.ascontiguousarray(Wo[sl, :]).astype(cdt),
            "bq2": np.ascontiguousarray(bq[sl]),
            "bk2": np.ascontiguousarray(bk[sl]),
            "bvb": np.ascontiguousarray(np.broadcast_to(bv[sl], (P, GD))),
            "maskd": mask,
        })

    trace = os.environ.get("KERNEL_TRACE", "0") == "1"
    res = run_bass_kernel_spmd(nc, in_maps, core_ids=list(range(8)), trace=trace)
    LAST_RESULTS = res

    out = np.zeros((B, S, D), np.float64)
    kp = np.empty((B, S, D), np.float32)
    vp = np.empty((B, S, D), np.float32)
    for core in range(8):
        b, g = divmod(core, 4)
        sl = slice(g * GD, (g + 1) * GD)
        r = res.results[core]
        kp[b][:, sl] = r["kpT"].astype(np.float32).T
        vp[b][:, sl] = r["vp"].astype(np.float32)
        out[b] += r["pout"].astype(np.float64)
    out = (out + bo.astype(np.float64)).astype(np.float32)
    return out, kp, vp
